# revision 4
# baseline (speedup 1.0000x reference)
"""Trainium2 Bass kernel for nn_EnsembleRatioModel.

Network (per row of x, N=131072 rows, fp32):
  trunk:   h0 = relu(x) @ W0 + b0 ; h1 = relu(h0) @ W1 + b1 ; repr = relu(h1) @ W2 + b2
  subnets: per pair p in {0,1,2}:
           s0 = relu(repr) @ SW0[p] + Sb0[p]
           s1 = relu(s0)   @ SW1[p] + Sb1[p]
           logit[p] = relu(s1) @ SW2[p] + Sb2[p]
  outputs: (repr [N,512], r_hat*mask [3,N], s_hat*mask [3,N], mask [3,N])

Sharding: data-parallel over the row dim across 8 cores (16384 rows/core).
Device layout: transposed activations — features on SBUF partitions, rows on
the free dim, so every layer is out.T = W.T @ in.T with the weight as the
stationary (lhsT) operand in its natural [in_dim, out_dim] layout.

Host does the cheap glue: relu+transpose+pad of x, the final sigmoid /
clip / ratio / mask over [3, N], and the repr transpose back to row-major.
"""

import numpy as np

import concourse.bass as bass
import concourse.bacc as bacc
import concourse.mybir as mybir
import concourse.tile as tile
from concourse.bass_utils import run_bass_kernel_spmd

F32 = mybir.dt.float32
F32R = mybir.dt.float32r

N_CORES = 8
N_ROWS = 131072
ROWS = N_ROWS // N_CORES        # 16384 rows per core
F = 512                         # rows per tile (matmul free dim, fp32 max)
NT = ROWS // F                  # 32 row tiles per core
D_IN = 64                       # observables
H = 512                         # trunk width
S0 = 512                        # subnet hidden 0
S1 = 256                        # subnet hidden 1
P = 3                           # pairs

# matmul input dtype: float32r streams 1 row/cycle (plain fp32 costs 4)
MM_DT = F32R


def _mm(ap):
    return ap


def _build_bass():
    nc = bacc.Bacc("TRN2", target_bir_lowering=False)

    xt = nc.dram_tensor("xt", [128, ROWS], F32R, kind="ExternalInput")
    w0 = nc.dram_tensor("w0", [128, H], F32R, kind="ExternalInput")
    w1 = nc.dram_tensor("w1", [H, H], F32R, kind="ExternalInput")
    w2 = nc.dram_tensor("w2", [H, H], F32R, kind="ExternalInput")
    sw0 = nc.dram_tensor("sw0", [P, H, S0], F32R, kind="ExternalInput")
    sw1 = nc.dram_tensor("sw1", [P, S0, S1], F32R, kind="ExternalInput")
    sw2 = nc.dram_tensor("sw2", [P * S1, P], F32R, kind="ExternalInput")  # blockdiag
    b0 = nc.dram_tensor("b0", [H], F32, kind="ExternalInput")
    b1 = nc.dram_tensor("b1", [H], F32, kind="ExternalInput")
    b2 = nc.dram_tensor("b2", [H], F32, kind="ExternalInput")
    sb0 = nc.dram_tensor("sb0", [P, S0], F32, kind="ExternalInput")
    sb1 = nc.dram_tensor("sb1", [P, S1], F32, kind="ExternalInput")

    reprT = nc.dram_tensor("reprT", [H, ROWS], F32, kind="ExternalOutput")
    logitT = nc.dram_tensor("logitT", [P, ROWS], F32, kind="ExternalOutput")

    reprT_v = reprT.rearrange("(mo ki) n -> ki mo n", ki=128)  # [128, 4, ROWS]

    ts = bass.ts

    with tile.TileContext(nc) as tc:
        with (
            tc.tile_pool(name="consts", bufs=1) as consts,
            tc.tile_pool(name="xin", bufs=3) as xin,
            tc.tile_pool(name="acts", bufs=2) as acts,
            tc.tile_pool(name="outs", bufs=2) as outs,
            tc.tile_pool(name="psum", bufs=6, space="PSUM") as psum,
            tc.tile_pool(name="psl", bufs=2, space="PSUM") as psl_pool,
        ):
            # --- resident weights/biases, features on partitions ---
            w0s = consts.tile([128, H], F32R)
            nc.sync.dma_start(w0s, w0[:, :])
            w1s = consts.tile([128, 4, H], F32R)
            nc.sync.dma_start(w1s, w1.rearrange("(ko ki) m -> ki ko m", ki=128))
            w2s = consts.tile([128, 4, H], F32R)
            nc.sync.dma_start(w2s, w2.rearrange("(ko ki) m -> ki ko m", ki=128))
            sw0s = consts.tile([128, P, 4, S0], F32R)
            nc.sync.dma_start(sw0s, sw0.rearrange("p (ko ki) m -> ki p ko m", ki=128))
            sw1s = consts.tile([128, P, 4, S1], F32R)
            nc.sync.dma_start(sw1s, sw1.rearrange("p (ko ki) m -> ki p ko m", ki=128))
            sw2s = consts.tile([128, 6, P], F32R)
            nc.sync.dma_start(sw2s, sw2.rearrange("(ko ki) m -> ki ko m", ki=128))
            b0s = consts.tile([128, 4], F32)
            nc.sync.dma_start(b0s, b0.rearrange("(ko ki) -> ki ko", ki=128))
            b1s = consts.tile([128, 4], F32)
            nc.sync.dma_start(b1s, b1.rearrange("(ko ki) -> ki ko", ki=128))
            b2s = consts.tile([128, 4], F32)
            nc.sync.dma_start(b2s, b2.rearrange("(ko ki) -> ki ko", ki=128))
            sb0s = consts.tile([128, P, 4], F32)
            nc.sync.dma_start(sb0s, sb0.rearrange("p (ko ki) -> ki p ko", ki=128))
            sb1s = consts.tile([128, P, 2], F32)
            nc.sync.dma_start(sb1s, sb1.rearrange("p (ko ki) -> ki p ko", ki=128))

            # alternate PSUM->SBUF epilogues between ScalarE and VectorE
            ew_ctr = [0]

            def ew(out, ps, bias_ap, relu):
                ew_ctr[0] += 1
                if ew_ctr[0] % 2 == 0:
                    if relu:
                        nc.scalar.activation(
                            out, ps, mybir.ActivationFunctionType.Relu, bias=bias_ap
                        )
                    else:
                        nc.scalar.activation(
                            out, ps, mybir.ActivationFunctionType.Identity, bias=bias_ap
                        )
                else:
                    if relu:
                        nc.vector.tensor_scalar(
                            out, ps, bias_ap, 0.0,
                            mybir.AluOpType.add, mybir.AluOpType.max,
                        )
                    else:
                        nc.vector.tensor_scalar_add(out, ps, bias_ap)

            for t in range(NT):
                cs = ts(t, F)

                xt_t = xin.tile([128, F], F32R, tag="xt_t")
                nc.sync.dma_start(xt_t, xt[:, cs])

                # trunk layer 0: K=128 (zero-padded from 64)
                h0r = acts.tile([128, 4, F], F32R, tag="h0r")
                for m in range(4):
                    ps = psum.tile([128, F], F32, tag="ps")
                    nc.tensor.matmul(
                        ps, _mm(w0s[:, ts(m, 128)]), _mm(xt_t), start=True, stop=True
                    )
                    ew(h0r[:, m], ps, b0s[:, m : m + 1], relu=True)

                # trunk layer 1
                h1r = acts.tile([128, 4, F], F32R, tag="h1r")
                for m in range(4):
                    ps = psum.tile([128, F], F32, tag="ps")
                    for k in range(4):
                        nc.tensor.matmul(
                            ps, _mm(w1s[:, k, ts(m, 128)]), _mm(h0r[:, k]),
                            start=(k == 0), stop=(k == 3),
                        )
                    ew(h1r[:, m], ps, b1s[:, m : m + 1], relu=True)

                # trunk layer 2 -> repr (raw out + relu'd for subnets)
                repr_raw = outs.tile([128, 4, F], F32, tag="repr_raw")
                repr_relu = acts.tile([128, 4, F], F32R, tag="repr_relu")
                for m in range(4):
                    ps = psum.tile([128, F], F32, tag="ps")
                    for k in range(4):
                        nc.tensor.matmul(
                            ps, _mm(w2s[:, k, ts(m, 128)]), _mm(h1r[:, k]),
                            start=(k == 0), stop=(k == 3),
                        )
                    ew(repr_raw[:, m], ps, b2s[:, m : m + 1], relu=False)
                    ew(repr_relu[:, m], ps, b2s[:, m : m + 1], relu=True)
                nc.sync.dma_start(reprT_v[:, :, cs], repr_raw)

                # subnets
                hs1r = acts.tile([128, 6, F], F32R, tag="hs1r")
                for p in range(P):
                    hs0r = acts.tile([128, 4, F], F32R, tag="hs0r")
                    for m in range(4):
                        ps = psum.tile([128, F], F32, tag="ps")
                        for k in range(4):
                            nc.tensor.matmul(
                                ps, _mm(sw0s[:, p, k, ts(m, 128)]), _mm(repr_relu[:, k]),
                                start=(k == 0), stop=(k == 3),
                            )
                        ew(hs0r[:, m], ps, sb0s[:, p, m : m + 1], relu=True)
                    for m in range(2):
                        ps = psum.tile([128, F], F32, tag="ps")
                        for k in range(4):
                            nc.tensor.matmul(
                                ps, _mm(sw1s[:, p, k, ts(m, 128)]), _mm(hs0r[:, k]),
                                start=(k == 0), stop=(k == 3),
                            )
                        ew(hs1r[:, 2 * p + m], ps, sb1s[:, p, m : m + 1], relu=True)

                # final layer, all 3 pairs at once via block-diagonal weights
                psl = psl_pool.tile([P, F], F32, tag="psl")
                for k in range(6):
                    nc.tensor.matmul(
                        psl, _mm(sw2s[:, k]), _mm(hs1r[:, k]),
                        start=(k == 0), stop=(k == 5),
                    )
                lg = outs.tile([P, F], F32, tag="lg")
                nc.vector.tensor_copy(lg, psl)
                nc.sync.dma_start(logitT[:, cs], lg)

    nc.finalize()
    return nc


_CACHED = {}


def _get_nc():
    if "nc" not in _CACHED:
        _CACHED["nc"] = _build_bass()
    return _CACHED["nc"]


def kernel(x, y, pairs, W0, b0, W1, b1, W2, b2, SW0, Sb0, SW1, Sb1, SW2, Sb2):
    x = np.asarray(x, dtype=np.float32)
    y_np = np.asarray(y)
    pairs_np = np.asarray(pairs)
    W0 = np.asarray(W0, np.float32); b0 = np.asarray(b0, np.float32)
    W1 = np.asarray(W1, np.float32); b1 = np.asarray(b1, np.float32)
    W2 = np.asarray(W2, np.float32); b2 = np.asarray(b2, np.float32)
    SW0 = np.asarray(SW0, np.float32); Sb0 = np.asarray(Sb0, np.float32)
    SW1 = np.asarray(SW1, np.float32); Sb1 = np.asarray(Sb1, np.float32)
    SW2 = np.asarray(SW2, np.float32); Sb2 = np.asarray(Sb2, np.float32)

    # host prep: relu(x), transpose to [features, rows], zero-pad 64 -> 128
    xr = np.maximum(x, 0.0)
    w0p = np.zeros((128, H), np.float32)
    w0p[:D_IN] = W0
    sw2bd = np.zeros((P * S1, P), np.float32)
    for p in range(P):
        sw2bd[p * S1 : (p + 1) * S1, p] = SW2[p, :, 0]

    in_maps = []
    for c in range(N_CORES):
        xt_c = np.zeros((128, ROWS), np.float32)
        xt_c[:D_IN] = xr[c * ROWS : (c + 1) * ROWS].T
        in_maps.append({
            "xt": xt_c, "w0": w0p, "w1": W1, "w2": W2,
            "sw0": SW0, "sw1": SW1, "sw2": sw2bd,
            "b0": b0, "b1": b1, "b2": b2, "sb0": Sb0, "sb1": Sb1,
        })

    nc = _get_nc()
    res = run_bass_kernel_spmd(nc, in_maps, core_ids=list(range(N_CORES)))

    repr_full = np.empty((N_ROWS, H), np.float32)
    logit = np.empty((P, N_ROWS), np.float32)
    for c in range(N_CORES):
        out = res.results[c]
        repr_full[c * ROWS : (c + 1) * ROWS] = out["reprT"].T
        logit[:, c * ROWS : (c + 1) * ROWS] = out["logitT"]

    # host post: bias, sigmoid, clip, ratio, mask
    logit += Sb2[:, 0:1]
    s_hat = 1.0 / (1.0 + np.exp(-logit, dtype=np.float32))
    s_hat = np.clip(s_hat, np.float32(1e-9), None)
    r_hat = (np.float32(1.0) - s_hat) / s_hat
    mask = (y_np[None, :] == pairs_np[:, 0:1]) | (y_np[None, :] == pairs_np[:, 1:2])
    maskf = mask.astype(np.float32)
    return repr_full, r_hat * maskf, s_hat * maskf, mask


# revision 8
# speedup vs baseline: 130.9851x; 130.9851x over previous
"""Trainium2 Bass kernel for nn_EnsembleRatioModel.

Network (per row of x, N=131072 rows, fp32):
  trunk:   h0 = relu(x) @ W0 + b0 ; h1 = relu(h0) @ W1 + b1 ; repr = relu(h1) @ W2 + b2
  subnets: per pair p in {0,1,2}:
           s0 = relu(repr) @ SW0[p] + Sb0[p]
           s1 = relu(s0)   @ SW1[p] + Sb1[p]
           logit[p] = relu(s1) @ SW2[p] + Sb2[p]
  outputs: (repr [N,512], r_hat*mask [3,N], s_hat*mask [3,N], mask [3,N])

Sharding: data-parallel over the row dim across 8 cores (16384 rows/core).
Device layout: transposed activations — features on SBUF partitions, rows on
the free dim, so every layer is out.T = W.T @ in.T with the weight as the
stationary (lhsT) operand in its natural [in_dim, out_dim] layout.

Host does the cheap glue: relu+transpose+pad of x, the final sigmoid /
clip / ratio / mask over [3, N], and the repr transpose back to row-major.
"""

import numpy as np

import concourse.bass as bass
import concourse.bacc as bacc
import concourse.mybir as mybir
import concourse.tile as tile
from concourse.bass_utils import run_bass_kernel_spmd

F32 = mybir.dt.float32
F32R = mybir.dt.float32r

N_CORES = 8
N_ROWS = 131072
ROWS = N_ROWS // N_CORES        # 16384 rows per core
F = 512                         # rows per tile (matmul free dim, fp32 max)
NT = ROWS // F                  # 32 row tiles per core
D_IN = 64                       # observables
H = 512                         # trunk width
S0 = 512                        # subnet hidden 0
S1 = 256                        # subnet hidden 1
P = 3                           # pairs

# matmul input dtype: float32r streams 1 row/cycle (plain fp32 costs 4)
MM_DT = F32R


def _mm(ap):
    return ap


def _build_bass(repeat=1, bufs=None):
    nc = bacc.Bacc("TRN2", target_bir_lowering=False)

    xt = nc.dram_tensor("xt", [128, ROWS], F32R, kind="ExternalInput")
    w0 = nc.dram_tensor("w0", [128, H], F32R, kind="ExternalInput")
    w1 = nc.dram_tensor("w1", [H, H], F32R, kind="ExternalInput")
    w2 = nc.dram_tensor("w2", [H, H], F32R, kind="ExternalInput")
    sw0 = nc.dram_tensor("sw0", [P, H, S0], F32R, kind="ExternalInput")
    sw1 = nc.dram_tensor("sw1", [P, S0, S1], F32R, kind="ExternalInput")
    sw2 = nc.dram_tensor("sw2", [P * S1, P], F32R, kind="ExternalInput")  # blockdiag
    b0 = nc.dram_tensor("b0", [H], F32, kind="ExternalInput")
    b1 = nc.dram_tensor("b1", [H], F32, kind="ExternalInput")
    b2 = nc.dram_tensor("b2", [H], F32, kind="ExternalInput")
    sb0 = nc.dram_tensor("sb0", [P, S0], F32, kind="ExternalInput")
    sb1 = nc.dram_tensor("sb1", [P, S1], F32, kind="ExternalInput")

    reprT = nc.dram_tensor("reprT", [H, ROWS], F32, kind="ExternalOutput")
    logitT = nc.dram_tensor("logitT", [P, ROWS], F32, kind="ExternalOutput")

    reprT_v = reprT.rearrange("(mo ki) n -> ki mo n", ki=128)  # [128, 4, ROWS]

    ts = bass.ts

    cfg = {"xin": 3, "h0r": 2, "h1r": 2, "repr_relu": 2, "hs0r": 2, "hs1r": 2,
           "repr_raw": 2, "lg": 2, "psum": 6, "psl": 2}
    if bufs:
        cfg.update(bufs)

    with tile.TileContext(nc) as tc:
        with (
            tc.tile_pool(name="consts", bufs=1) as consts,
            tc.tile_pool(name="xin", bufs=cfg["xin"]) as xin,
            tc.tile_pool(name="p_h0r", bufs=cfg["h0r"]) as p_h0r,
            tc.tile_pool(name="p_h1r", bufs=cfg["h1r"]) as p_h1r,
            tc.tile_pool(name="p_rrelu", bufs=cfg["repr_relu"]) as p_rrelu,
            tc.tile_pool(name="p_hs0r", bufs=cfg["hs0r"]) as p_hs0r,
            tc.tile_pool(name="p_hs1r", bufs=cfg["hs1r"]) as p_hs1r,
            tc.tile_pool(name="p_rraw", bufs=cfg["repr_raw"]) as p_rraw,
            tc.tile_pool(name="p_lg", bufs=cfg["lg"]) as p_lg,
            tc.tile_pool(name="psum", bufs=cfg["psum"], space="PSUM") as psum,
            tc.tile_pool(name="psl", bufs=cfg["psl"], space="PSUM") as psl_pool,
        ):
            # --- resident weights/biases, features on partitions ---
            # biases + small tensors first so early tiles' epilogues don't
            # stall behind the bulk weight transfers
            b0s = consts.tile([128, 4], F32)
            nc.sync.dma_start(b0s, b0.rearrange("(ko ki) -> ki ko", ki=128))
            b1s = consts.tile([128, 4], F32)
            nc.sync.dma_start(b1s, b1.rearrange("(ko ki) -> ki ko", ki=128))
            b2s = consts.tile([128, 4], F32)
            nc.sync.dma_start(b2s, b2.rearrange("(ko ki) -> ki ko", ki=128))
            sb0s = consts.tile([128, P, 4], F32)
            nc.sync.dma_start(sb0s, sb0.rearrange("p (ko ki) -> ki p ko", ki=128))
            sb1s = consts.tile([128, P, 2], F32)
            nc.sync.dma_start(sb1s, sb1.rearrange("p (ko ki) -> ki p ko", ki=128))
            sw2s = consts.tile([128, 6, P], F32R)
            nc.sync.dma_start(sw2s, sw2.rearrange("(ko ki) m -> ki ko m", ki=128))
            w0s = consts.tile([128, H], F32R)
            nc.sync.dma_start(w0s, w0[:, :])
            w1s = consts.tile([128, 4, H], F32R)
            nc.sync.dma_start(w1s, w1.rearrange("(ko ki) m -> ki ko m", ki=128))
            w2s = consts.tile([128, 4, H], F32R)
            nc.sync.dma_start(w2s, w2.rearrange("(ko ki) m -> ki ko m", ki=128))
            sw0s = consts.tile([128, P, 4, S0], F32R)
            nc.sync.dma_start(sw0s, sw0.rearrange("p (ko ki) m -> ki p ko m", ki=128))
            sw1s = consts.tile([128, P, 4, S1], F32R)
            nc.sync.dma_start(sw1s, sw1.rearrange("p (ko ki) m -> ki p ko m", ki=128))

            # alternate PSUM->SBUF epilogues between ScalarE and VectorE
            ew_ctr = [0]

            def ew(out, ps, bias_ap, relu):
                ew_ctr[0] += 1
                if ew_ctr[0] % 2 == 0:
                    if relu:
                        nc.scalar.activation(
                            out, ps, mybir.ActivationFunctionType.Relu, bias=bias_ap
                        )
                    else:
                        nc.scalar.activation(
                            out, ps, mybir.ActivationFunctionType.Identity, bias=bias_ap
                        )
                else:
                    if relu:
                        nc.vector.tensor_scalar(
                            out, ps, bias_ap, 0.0,
                            mybir.AluOpType.add, mybir.AluOpType.max,
                        )
                    else:
                        nc.vector.tensor_scalar_add(out, ps, bias_ap)

            for t in list(range(NT)) * repeat:
                cs = ts(t, F)

                xt_t = xin.tile([128, F], F32R, tag="xt_t")
                nc.sync.dma_start(xt_t, xt[:, cs])

                # trunk layer 0: K=128 (zero-padded from 64)
                h0r = p_h0r.tile([128, 4, F], F32R, tag="h0r")
                for m in range(4):
                    ps = psum.tile([128, F], F32, tag="ps")
                    nc.tensor.matmul(
                        ps, _mm(w0s[:, ts(m, 128)]), _mm(xt_t), start=True, stop=True
                    )
                    ew(h0r[:, m], ps, b0s[:, m : m + 1], relu=True)

                # trunk layer 1
                h1r = p_h1r.tile([128, 4, F], F32R, tag="h1r")
                for m in range(4):
                    ps = psum.tile([128, F], F32, tag="ps")
                    for k in range(4):
                        nc.tensor.matmul(
                            ps, _mm(w1s[:, k, ts(m, 128)]), _mm(h0r[:, k]),
                            start=(k == 0), stop=(k == 3),
                        )
                    ew(h1r[:, m], ps, b1s[:, m : m + 1], relu=True)

                # trunk layer 2 -> repr (raw out + relu'd for subnets)
                repr_raw = p_rraw.tile([128, 4, F], F32, tag="repr_raw")
                repr_relu = p_rrelu.tile([128, 4, F], F32R, tag="repr_relu")
                for m in range(4):
                    ps = psum.tile([128, F], F32, tag="ps")
                    for k in range(4):
                        nc.tensor.matmul(
                            ps, _mm(w2s[:, k, ts(m, 128)]), _mm(h1r[:, k]),
                            start=(k == 0), stop=(k == 3),
                        )
                    ew(repr_raw[:, m], ps, b2s[:, m : m + 1], relu=False)
                    ew(repr_relu[:, m], ps, b2s[:, m : m + 1], relu=True)
                nc.sync.dma_start(reprT_v[:, :, cs], repr_raw)

                # subnets
                hs1r = p_hs1r.tile([128, 6, F], F32R, tag="hs1r")
                for p in range(P):
                    hs0r = p_hs0r.tile([128, 4, F], F32R, tag="hs0r")
                    for m in range(4):
                        ps = psum.tile([128, F], F32, tag="ps")
                        for k in range(4):
                            nc.tensor.matmul(
                                ps, _mm(sw0s[:, p, k, ts(m, 128)]), _mm(repr_relu[:, k]),
                                start=(k == 0), stop=(k == 3),
                            )
                        ew(hs0r[:, m], ps, sb0s[:, p, m : m + 1], relu=True)
                    for m in range(2):
                        ps = psum.tile([128, F], F32, tag="ps")
                        for k in range(4):
                            nc.tensor.matmul(
                                ps, _mm(sw1s[:, p, k, ts(m, 128)]), _mm(hs0r[:, k]),
                                start=(k == 0), stop=(k == 3),
                            )
                        ew(hs1r[:, 2 * p + m], ps, sb1s[:, p, m : m + 1], relu=True)

                # final layer, all 3 pairs at once via block-diagonal weights
                psl = psl_pool.tile([P, F], F32, tag="psl")
                for k in range(6):
                    nc.tensor.matmul(
                        psl, _mm(sw2s[:, k]), _mm(hs1r[:, k]),
                        start=(k == 0), stop=(k == 5),
                    )
                lg = p_lg.tile([P, F], F32, tag="lg")
                nc.vector.tensor_copy(lg, psl)
                nc.sync.dma_start(logitT[:, cs], lg)

    nc.finalize()
    return nc


_CACHED = {}


def _get_nc():
    if "nc" not in _CACHED:
        _CACHED["nc"] = _build_bass()
    return _CACHED["nc"]


def kernel(x, y, pairs, W0, b0, W1, b1, W2, b2, SW0, Sb0, SW1, Sb1, SW2, Sb2):
    x = np.asarray(x, dtype=np.float32)
    y_np = np.asarray(y)
    pairs_np = np.asarray(pairs)
    W0 = np.asarray(W0, np.float32); b0 = np.asarray(b0, np.float32)
    W1 = np.asarray(W1, np.float32); b1 = np.asarray(b1, np.float32)
    W2 = np.asarray(W2, np.float32); b2 = np.asarray(b2, np.float32)
    SW0 = np.asarray(SW0, np.float32); Sb0 = np.asarray(Sb0, np.float32)
    SW1 = np.asarray(SW1, np.float32); Sb1 = np.asarray(Sb1, np.float32)
    SW2 = np.asarray(SW2, np.float32); Sb2 = np.asarray(Sb2, np.float32)

    # host prep: relu(x), transpose to [features, rows], zero-pad 64 -> 128
    xr = np.maximum(x, 0.0)
    w0p = np.zeros((128, H), np.float32)
    w0p[:D_IN] = W0
    sw2bd = np.zeros((P * S1, P), np.float32)
    for p in range(P):
        sw2bd[p * S1 : (p + 1) * S1, p] = SW2[p, :, 0]

    in_maps = []
    for c in range(N_CORES):
        xt_c = np.zeros((128, ROWS), np.float32)
        xt_c[:D_IN] = xr[c * ROWS : (c + 1) * ROWS].T
        in_maps.append({
            "xt": xt_c, "w0": w0p, "w1": W1, "w2": W2,
            "sw0": SW0, "sw1": SW1, "sw2": sw2bd,
            "b0": b0, "b1": b1, "b2": b2, "sb0": Sb0, "sb1": Sb1,
        })

    nc = _get_nc()
    res = run_bass_kernel_spmd(nc, in_maps, core_ids=list(range(N_CORES)))

    repr_full = np.empty((N_ROWS, H), np.float32)
    logit = np.empty((P, N_ROWS), np.float32)
    for c in range(N_CORES):
        out = res.results[c]
        repr_full[c * ROWS : (c + 1) * ROWS] = out["reprT"].T
        logit[:, c * ROWS : (c + 1) * ROWS] = out["logitT"]

    # host post: bias, sigmoid, clip, ratio, mask
    logit += Sb2[:, 0:1]
    s_hat = 1.0 / (1.0 + np.exp(-logit, dtype=np.float32))
    s_hat = np.clip(s_hat, np.float32(1e-9), None)
    r_hat = (np.float32(1.0) - s_hat) / s_hat
    mask = (y_np[None, :] == pairs_np[:, 0:1]) | (y_np[None, :] == pairs_np[:, 1:2])
    maskf = mask.astype(np.float32)
    return repr_full, r_hat * maskf, s_hat * maskf, mask


# revision 11
# speedup vs baseline: 131.8552x; 1.0066x over previous
"""Trainium2 Bass kernel for nn_EnsembleRatioModel.

Network (per row of x, N=131072 rows, fp32):
  trunk:   h0 = relu(x) @ W0 + b0 ; h1 = relu(h0) @ W1 + b1 ; repr = relu(h1) @ W2 + b2
  subnets: per pair p in {0,1,2}:
           s0 = relu(repr) @ SW0[p] + Sb0[p]
           s1 = relu(s0)   @ SW1[p] + Sb1[p]
           logit[p] = relu(s1) @ SW2[p] + Sb2[p]
  outputs: (repr [N,512], r_hat*mask [3,N], s_hat*mask [3,N], mask [3,N])

Sharding: data-parallel over the row dim across 8 cores (16384 rows/core).
Device layout: transposed activations — features on SBUF partitions, rows on
the free dim, so every layer is out.T = W.T @ in.T with the weight as the
stationary (lhsT) operand in its natural [in_dim, out_dim] layout.

Host does the cheap glue: relu+transpose+pad of x, the final sigmoid /
clip / ratio / mask over [3, N], and the repr transpose back to row-major.
"""

import numpy as np

import concourse.bass as bass
import concourse.bacc as bacc
import concourse.mybir as mybir
import concourse.tile as tile
from concourse.bass_utils import run_bass_kernel_spmd

F32 = mybir.dt.float32
F32R = mybir.dt.float32r

N_CORES = 8
N_ROWS = 131072
ROWS = N_ROWS // N_CORES        # 16384 rows per core
F = 512                         # rows per tile (matmul free dim, fp32 max)
NT = ROWS // F                  # 32 row tiles per core
D_IN = 64                       # observables
H = 512                         # trunk width
S0 = 512                        # subnet hidden 0
S1 = 256                        # subnet hidden 1
P = 3                           # pairs

# matmul input dtype: float32r streams 1 row/cycle (plain fp32 costs 4)
MM_DT = F32R


def _mm(ap):
    return ap


def _build_bass(repeat=1, bufs=None):
    nc = bacc.Bacc("TRN2", target_bir_lowering=False)

    xt = nc.dram_tensor("xt", [128, ROWS], F32R, kind="ExternalInput")
    w0 = nc.dram_tensor("w0", [128, H], F32R, kind="ExternalInput")
    w1 = nc.dram_tensor("w1", [H, H], F32R, kind="ExternalInput")
    w2 = nc.dram_tensor("w2", [H, H], F32R, kind="ExternalInput")
    sw0 = nc.dram_tensor("sw0", [P, H, S0], F32R, kind="ExternalInput")
    sw1 = nc.dram_tensor("sw1", [P, S0, S1], F32R, kind="ExternalInput")
    sw2 = nc.dram_tensor("sw2", [P * S1, P], F32R, kind="ExternalInput")  # blockdiag
    b0 = nc.dram_tensor("b0", [H], F32, kind="ExternalInput")
    b1 = nc.dram_tensor("b1", [H], F32, kind="ExternalInput")
    b2 = nc.dram_tensor("b2", [H], F32, kind="ExternalInput")
    sb0 = nc.dram_tensor("sb0", [P, S0], F32, kind="ExternalInput")
    sb1 = nc.dram_tensor("sb1", [P, S1], F32, kind="ExternalInput")

    reprT = nc.dram_tensor("reprT", [H, ROWS], F32, kind="ExternalOutput")
    logitT = nc.dram_tensor("logitT", [P, ROWS], F32, kind="ExternalOutput")

    reprT_v = reprT.rearrange("(mo ki) n -> ki mo n", ki=128)  # [128, 4, ROWS]

    ts = bass.ts

    cfg = {"xin": 3, "h0r": 2, "h1r": 2, "repr_relu": 2, "hs0r": 2, "hs1r": 2,
           "repr_raw": 2, "lg": 2, "psum": 6, "psl": 2}
    if bufs:
        cfg.update(bufs)

    with tile.TileContext(nc) as tc:
        with (
            tc.tile_pool(name="consts", bufs=1) as consts,
            tc.tile_pool(name="xin", bufs=cfg["xin"]) as xin,
            tc.tile_pool(name="p_h0r", bufs=cfg["h0r"]) as p_h0r,
            tc.tile_pool(name="p_h1r", bufs=cfg["h1r"]) as p_h1r,
            tc.tile_pool(name="p_rrelu", bufs=cfg["repr_relu"]) as p_rrelu,
            tc.tile_pool(name="p_hs0r", bufs=cfg["hs0r"]) as p_hs0r,
            tc.tile_pool(name="p_hs1r", bufs=cfg["hs1r"]) as p_hs1r,
            tc.tile_pool(name="p_rraw", bufs=cfg["repr_raw"]) as p_rraw,
            tc.tile_pool(name="p_lg", bufs=cfg["lg"]) as p_lg,
            tc.tile_pool(name="psum", bufs=cfg["psum"], space="PSUM") as psum,
            tc.tile_pool(name="psl", bufs=cfg["psl"], space="PSUM") as psl_pool,
        ):
            # --- resident weights/biases, features on partitions ---
            # biases + small tensors first so early tiles' epilogues don't
            # stall behind the bulk weight transfers
            b0s = consts.tile([128, 4], F32)
            nc.sync.dma_start(b0s, b0.rearrange("(ko ki) -> ki ko", ki=128))
            b1s = consts.tile([128, 4], F32)
            nc.sync.dma_start(b1s, b1.rearrange("(ko ki) -> ki ko", ki=128))
            b2s = consts.tile([128, 4], F32)
            nc.sync.dma_start(b2s, b2.rearrange("(ko ki) -> ki ko", ki=128))
            sb0s = consts.tile([128, P, 4], F32)
            nc.sync.dma_start(sb0s, sb0.rearrange("p (ko ki) -> ki p ko", ki=128))
            sb1s = consts.tile([128, P, 2], F32)
            nc.sync.dma_start(sb1s, sb1.rearrange("p (ko ki) -> ki p ko", ki=128))
            sw2s = consts.tile([128, 6, P], F32R)
            nc.sync.dma_start(sw2s, sw2.rearrange("(ko ki) m -> ki ko m", ki=128))
            w0s = consts.tile([128, H], F32R)
            nc.sync.dma_start(w0s, w0[:, :])
            # prefetch the first x tiles ahead of the bulk weight DMAs so
            # layer-0 matmuls can start while subnet weights stream in
            xt_pre = {}
            for t0 in range(min(3, NT)):
                xt_pre[t0] = xin.tile([128, F], F32R, tag="xt_t", name=f"xt_pre{t0}")
                nc.sync.dma_start(xt_pre[t0], xt[:, ts(t0, F)])
            w1s = consts.tile([128, 4, H], F32R)
            nc.sync.dma_start(w1s, w1.rearrange("(ko ki) m -> ki ko m", ki=128))
            w2s = consts.tile([128, 4, H], F32R)
            nc.sync.dma_start(w2s, w2.rearrange("(ko ki) m -> ki ko m", ki=128))
            sw0s = consts.tile([128, P, 4, S0], F32R)
            nc.sync.dma_start(sw0s, sw0.rearrange("p (ko ki) m -> ki p ko m", ki=128))
            sw1s = consts.tile([128, P, 4, S1], F32R)
            nc.sync.dma_start(sw1s, sw1.rearrange("p (ko ki) m -> ki p ko m", ki=128))

            # alternate PSUM->SBUF epilogues between ScalarE and VectorE
            ew_ctr = [0]

            def ew(out, ps, bias_ap, relu):
                ew_ctr[0] += 1
                if ew_ctr[0] % 2 == 0:
                    if relu:
                        nc.scalar.activation(
                            out, ps, mybir.ActivationFunctionType.Relu, bias=bias_ap
                        )
                    else:
                        nc.scalar.activation(
                            out, ps, mybir.ActivationFunctionType.Identity, bias=bias_ap
                        )
                else:
                    if relu:
                        nc.vector.tensor_scalar(
                            out, ps, bias_ap, 0.0,
                            mybir.AluOpType.add, mybir.AluOpType.max,
                        )
                    else:
                        nc.vector.tensor_scalar_add(out, ps, bias_ap)

            for t in list(range(NT)) * repeat:
                cs = ts(t, F)

                if t in xt_pre:
                    xt_t = xt_pre.pop(t)
                else:
                    xt_t = xin.tile([128, F], F32R, tag="xt_t")
                    nc.sync.dma_start(xt_t, xt[:, cs])

                # trunk layer 0: K=128 (zero-padded from 64)
                h0r = p_h0r.tile([128, 4, F], F32R, tag="h0r")
                for m in range(4):
                    ps = psum.tile([128, F], F32, tag="ps")
                    nc.tensor.matmul(
                        ps, _mm(w0s[:, ts(m, 128)]), _mm(xt_t), start=True, stop=True
                    )
                    ew(h0r[:, m], ps, b0s[:, m : m + 1], relu=True)

                # trunk layer 1
                h1r = p_h1r.tile([128, 4, F], F32R, tag="h1r")
                for m in range(4):
                    ps = psum.tile([128, F], F32, tag="ps")
                    for k in range(4):
                        nc.tensor.matmul(
                            ps, _mm(w1s[:, k, ts(m, 128)]), _mm(h0r[:, k]),
                            start=(k == 0), stop=(k == 3),
                        )
                    ew(h1r[:, m], ps, b1s[:, m : m + 1], relu=True)

                # trunk layer 2 -> repr (raw out + relu'd for subnets)
                repr_raw = p_rraw.tile([128, 4, F], F32, tag="repr_raw")
                repr_relu = p_rrelu.tile([128, 4, F], F32R, tag="repr_relu")
                for m in range(4):
                    ps = psum.tile([128, F], F32, tag="ps")
                    for k in range(4):
                        nc.tensor.matmul(
                            ps, _mm(w2s[:, k, ts(m, 128)]), _mm(h1r[:, k]),
                            start=(k == 0), stop=(k == 3),
                        )
                    ew(repr_raw[:, m], ps, b2s[:, m : m + 1], relu=False)
                    ew(repr_relu[:, m], ps, b2s[:, m : m + 1], relu=True)
                nc.sync.dma_start(reprT_v[:, :, cs], repr_raw)

                # subnets
                hs1r = p_hs1r.tile([128, 6, F], F32R, tag="hs1r")
                for p in range(P):
                    hs0r = p_hs0r.tile([128, 4, F], F32R, tag="hs0r")
                    for m in range(4):
                        ps = psum.tile([128, F], F32, tag="ps")
                        for k in range(4):
                            nc.tensor.matmul(
                                ps, _mm(sw0s[:, p, k, ts(m, 128)]), _mm(repr_relu[:, k]),
                                start=(k == 0), stop=(k == 3),
                            )
                        ew(hs0r[:, m], ps, sb0s[:, p, m : m + 1], relu=True)
                    for m in range(2):
                        ps = psum.tile([128, F], F32, tag="ps")
                        for k in range(4):
                            nc.tensor.matmul(
                                ps, _mm(sw1s[:, p, k, ts(m, 128)]), _mm(hs0r[:, k]),
                                start=(k == 0), stop=(k == 3),
                            )
                        ew(hs1r[:, 2 * p + m], ps, sb1s[:, p, m : m + 1], relu=True)

                # final layer, all 3 pairs at once via block-diagonal weights
                psl = psl_pool.tile([P, F], F32, tag="psl")
                for k in range(6):
                    nc.tensor.matmul(
                        psl, _mm(sw2s[:, k]), _mm(hs1r[:, k]),
                        start=(k == 0), stop=(k == 5),
                    )
                lg = p_lg.tile([P, F], F32, tag="lg")
                nc.vector.tensor_copy(lg, psl)
                nc.sync.dma_start(logitT[:, cs], lg)

    nc.finalize()
    return nc


_CACHED = {}


def _get_nc():
    if "nc" not in _CACHED:
        _CACHED["nc"] = _build_bass()
    return _CACHED["nc"]


def kernel(x, y, pairs, W0, b0, W1, b1, W2, b2, SW0, Sb0, SW1, Sb1, SW2, Sb2):
    x = np.asarray(x, dtype=np.float32)
    y_np = np.asarray(y)
    pairs_np = np.asarray(pairs)
    W0 = np.asarray(W0, np.float32); b0 = np.asarray(b0, np.float32)
    W1 = np.asarray(W1, np.float32); b1 = np.asarray(b1, np.float32)
    W2 = np.asarray(W2, np.float32); b2 = np.asarray(b2, np.float32)
    SW0 = np.asarray(SW0, np.float32); Sb0 = np.asarray(Sb0, np.float32)
    SW1 = np.asarray(SW1, np.float32); Sb1 = np.asarray(Sb1, np.float32)
    SW2 = np.asarray(SW2, np.float32); Sb2 = np.asarray(Sb2, np.float32)

    # host prep: relu(x), transpose to [features, rows], zero-pad 64 -> 128
    xr = np.maximum(x, 0.0)
    w0p = np.zeros((128, H), np.float32)
    w0p[:D_IN] = W0
    sw2bd = np.zeros((P * S1, P), np.float32)
    for p in range(P):
        sw2bd[p * S1 : (p + 1) * S1, p] = SW2[p, :, 0]

    in_maps = []
    for c in range(N_CORES):
        xt_c = np.zeros((128, ROWS), np.float32)
        xt_c[:D_IN] = xr[c * ROWS : (c + 1) * ROWS].T
        in_maps.append({
            "xt": xt_c, "w0": w0p, "w1": W1, "w2": W2,
            "sw0": SW0, "sw1": SW1, "sw2": sw2bd,
            "b0": b0, "b1": b1, "b2": b2, "sb0": Sb0, "sb1": Sb1,
        })

    nc = _get_nc()
    res = run_bass_kernel_spmd(nc, in_maps, core_ids=list(range(N_CORES)))

    repr_full = np.empty((N_ROWS, H), np.float32)
    logit = np.empty((P, N_ROWS), np.float32)
    for c in range(N_CORES):
        out = res.results[c]
        repr_full[c * ROWS : (c + 1) * ROWS] = out["reprT"].T
        logit[:, c * ROWS : (c + 1) * ROWS] = out["logitT"]

    # host post: bias, sigmoid, clip, ratio, mask
    logit += Sb2[:, 0:1]
    z = np.exp(-np.abs(logit))
    s_hat = np.where(logit >= 0, 1.0 / (1.0 + z), z / (1.0 + z)).astype(np.float32)
    s_hat = np.clip(s_hat, np.float32(1e-9), None)
    r_hat = (np.float32(1.0) - s_hat) / s_hat
    mask = (y_np[None, :] == pairs_np[:, 0:1]) | (y_np[None, :] == pairs_np[:, 1:2])
    maskf = mask.astype(np.float32)
    return repr_full, r_hat * maskf, s_hat * maskf, mask
